# revision 11
# baseline (speedup 1.0000x reference)
"""Expert-parallel SwiGLU MLP (MoE experts) for 8 Trainium2 NeuronCores.

Problem: routed_in_egD [E*G, D] fp32, w1/w3 [E, D, F], w2 [E, F, D], E=8,
G=2048, D=2048, F=5632.  reference:
    x_egD = routed.reshape(E, G, D)
    mid   = silu(x @ w1) * (x @ w3)          # [E, G, F]
    out   = (mid @ w2).reshape(E*G, D)

Sharding: expert-parallel - core e gets expert e's x slice + weights; no
collectives.  Each core runs three 2048x2048x5632-class GEMMs (~142 GFLOP),
matmul-roofline-bound at the bf16 PE rate (1 col/cycle @ 2.4 GHz -> ~1.80 ms
of pure matmul streaming per core), so the whole game is keeping the PE at
~100% MATMUL occupancy: no PE transposes, no phase-boundary stalls.

Per-core kernel (all matmuls bf16, PSUM fp32):
  phase A: SWDGE casts x -> xbf (DRAM, bf16) in 4 g-blocks; HWDGE xbar
           DMA-transposes each block DRAM->SBUF into its own contiguous tile
           xTb[b] [P, DO, 512] (d = do*128 + p row mapping).  SBUF->SBUF
           transposes are NOT used (Tile's deadlock-avoidance serializes them
           to ~10us each); DRAM-sourced transposes run at full rate.  The PE
           does zero transpose work.
  phase 1: gh-outer (g-halves): per (gh, fc): gateT/upT = w1/w3.T @ x
           accumulated over d in PSUM (j-outer so the first 32 matmuls need
           only xTb[0]); SwiGLU (ACT silu -> bf16, DVE mul); midT spilled to
           DRAM bf16 per-gp tiles => precise DMA deps.  w1/w3 are re-read per
           gh (DMA has headroom; PE does not).  w2 dq=0 panel is cast-loaded
           into a pre-allocated buffer DURING gh=0 (interleaved on the SWDGE
           queue), and mq panels for early gp prefetch on the idle sync queue,
           so phase 2 starts with zero DMA wait.
  phase 2: out[g,d] = sum_f midT[f,g]*w2[f,d]: mid panels stationary (bf16),
           w2 panels DMA-cast fp32->bf16 (moving), PSUM accumulation over F.
"""

import numpy as np

import concourse.mybir as mybir
import concourse.tile as tile
from concourse import bacc
from concourse.bass_utils import run_bass_kernel_spmd

E, G, D, F = 8, 2048, 2048, 5632
P = 128
DO = D // P      # 16 d-chunks (contraction steps)
FC = F // P      # 44 f-chunks
GB = 4           # g-blocks of 512 for the x transpose

F32 = mybir.dt.float32
BF16 = mybir.dt.bfloat16


def build_nc():
    nc = bacc.Bacc("TRN2", target_bir_lowering=False)
    x = nc.dram_tensor("x", [G, D], F32, kind="ExternalInput").ap()
    w1 = nc.dram_tensor("w1", [D, F], F32, kind="ExternalInput").ap()
    w2 = nc.dram_tensor("w2", [F, D], F32, kind="ExternalInput").ap()
    w3 = nc.dram_tensor("w3", [D, F], F32, kind="ExternalInput").ap()
    out = nc.dram_tensor("out", [G, D], F32, kind="ExternalOutput").ap()

    # xbar transpose of xbf[g-block, :] into xTb[b] [p, do, g] maps rows as
    # d = do*128 + p (natural d-chunking: partition p within chunk do).
    w1r = w1.rearrange("(do p) f -> p do f", p=P)
    w3r = w3.rearrange("(do p) f -> p do f", p=P)
    w2r = w2.rearrange("(fo p) d -> p fo d", p=P)

    with tile.TileContext(nc) as tc:
        dram = tc.alloc_tile_pool(name="dram", bufs=1, space="DRAM")
        xbf = dram.tile([G, D], BF16, name="xbf")
        # per-gp mid tiles: mids[gp][p, fo, g'] = mid for f = fo*128+p,
        # g = gp*256 + g'.  Phase-1 writes are per-partition contiguous 512B;
        # phase-2 panel reads are per-partition contiguous 22KB.
        mids = [
            dram.tile([P, FC, 256], BF16, tag=f"mid{gp}", name=f"mid{gp}")
            for gp in range(8)
        ]

        # right-side pools pre-allocated so phase-2 inputs stream in during
        # phase 1 (their SBUF space never overlaps the phase-1 pools)
        mqp = tc.alloc_tile_pool(name="mqp", bufs=2, side="right")
        w2p0 = tc.alloc_tile_pool(name="w2p0", bufs=1, side="right")
        w2q0 = w2p0.tile([P, FC, 512], BF16, name="w2q0")

        xtp = tc.alloc_tile_pool(name="xtp", bufs=1)
        xTb = [
            xtp.tile([P, DO, 512], BF16, tag=f"xTb{b}", name=f"xTb{b}")
            for b in range(GB)
        ]

        # ---- phase A: stage x as bf16 in DRAM and xbar-transpose per g-block.
        # HWDGE load (sync) -> DVE cast -> HWDGE store (scalar) -> HWDGE
        # transpose (sync).  The SWDGE queue is left entirely to the w1/w3
        # weight stream (its cast path runs at only ~100-200 GB/s and the
        # first weight tiles gate the first matmuls).
        xfp = tc.alloc_tile_pool(name="xfp", bufs=2)
        xcp = tc.alloc_tile_pool(name="xcp", bufs=2)
        for b in range(GB):
            for s in range(4):
                r = b * 4 + s
                xf = xfp.tile([P, D], F32, tag="xf")
                nc.sync.dma_start(xf, x[r * P : (r + 1) * P, :])
                xc = xcp.tile([P, D], BF16, tag="xc")
                nc.vector.tensor_copy(xc, xf)
                nc.scalar.dma_start(xbf[r * P : (r + 1) * P, :], xc)
            nc.sync.dma_start_transpose(xTb[b], xbf[b * 512 : (b + 1) * 512, :])

        # ---- phase 1: midT[f, g] = silu(w1.T x) * (w3.T x), spill bf16
        wp = tc.alloc_tile_pool(name="wp", bufs=3)
        mp = tc.alloc_tile_pool(name="mp", bufs=6)
        ps1g = tc.alloc_tile_pool(name="ps1g", bufs=4, space="PSUM")
        ps1u = tc.alloc_tile_pool(name="ps1u", bufs=4, space="PSUM")
        w2bounds = [0, 6, 12, 18, 24, 29, 34, 39, 44]
        for gh in range(2):
            # per-(fc, j) accumulation groups.  For gh=0 the first two
            # f-chunks run j-major so the PE only needs xTb[0] for the first
            # 64 matmuls (xTb[1] is still being transposed).
            if gh == 0:
                pairs = [(0, 0), (1, 0), (0, 1), (1, 1)]
            else:
                pairs = [(0, 0), (0, 1), (1, 0), (1, 1)]
            pairs += [(fc, j) for fc in range(2, FC) for j in range(2)]
            wts = {}
            for fc, j in pairs:
                if j == 0 and fc not in wts:
                    w1t = wp.tile([P, DO, P], BF16, tag="w1", name=f"w1t{fc}")
                    nc.gpsimd.dma_start(w1t, w1r[:, :, fc * P : (fc + 1) * P])
                    w3t = wp.tile([P, DO, P], BF16, tag="w3", name=f"w3t{fc}")
                    nc.gpsimd.dma_start(w3t, w3r[:, :, fc * P : (fc + 1) * P])
                    wts = {fc: (w1t, w3t), **{k: v for k, v in wts.items() if k >= fc - 1}}
                    if gh == 0 and 4 <= fc < 12:
                        # stream the dq=0 w2 panel into its pre-allocated
                        # buffer while the SWDGE queue has slack
                        lo, hi = w2bounds[fc - 4], w2bounds[fc - 3]
                        nc.gpsimd.dma_start(
                            w2q0[:, lo:hi, :], w2r[:, lo:hi, 0:512]
                        )
                w1t, w3t = wts[fc]
                xs = xTb[gh * 2 + j]
                pg = ps1g.tile([P, 512], F32, tag="pg")
                pu = ps1u.tile([P, 512], F32, tag="pu")
                for d in range(DO):
                    st, sp_ = (d == 0), (d == DO - 1)
                    nc.tensor.matmul(pg, w1t[:, d], xs[:, d], start=st, stop=sp_)
                    nc.tensor.matmul(pu, w3t[:, d], xs[:, d], start=st, stop=sp_)
                mo = mp.tile([P, 2, 256], BF16, tag="mo")
                nc.scalar.activation(
                    mo, pg, mybir.ActivationFunctionType.Silu
                )
                nc.vector.tensor_mul(mo, mo, pu)
                for k in range(2):
                    nc.scalar.dma_start(
                        mids[gh * 4 + j * 2 + k][:, fc], mo[:, k]
                    )
        mp.release()
        wp.release()
        xcp.release()
        xfp.release()
        xtp.release()
        ps1u.release()
        ps1g.release()

        # ---- phase 2: out[g, d] = midT.T @ w2 (bf16 x bf16, fp32 psum)
        w2p = tc.alloc_tile_pool(name="w2p", bufs=2, side="right")
        op = tc.alloc_tile_pool(name="op", bufs=4, side="right")
        ps2 = tc.alloc_tile_pool(name="ps2", bufs=3, space="PSUM")
        for dq in range(4):
            if dq == 0:
                w2q = w2q0
            else:
                w2q = w2p.tile([P, FC, 512], BF16, tag="w2q")
                nc.gpsimd.dma_start(w2q, w2r[:, :, dq * 512 : (dq + 1) * 512])
            for gp in range(8):
                mq = mqp.tile([P, FC, 256], BF16, tag="mq")
                dma_eng = nc.sync if gp % 2 == 0 else nc.scalar
                dma_eng.dma_start(mq, mids[gp])
                po = ps2.tile([P, 2, 512], F32, tag="po")
                for fo in range(FC):
                    st, sp_ = (fo == 0), (fo == FC - 1)
                    for gc in range(2):
                        nc.tensor.matmul(
                            po[:, gc],
                            mq[:, fo, gc * P : (gc + 1) * P],
                            w2q[:, fo],
                            start=st,
                            stop=sp_,
                        )
                for gc in range(2):
                    ot = op.tile([P, 512], F32, tag="ot")
                    nc.vector.tensor_copy(ot, po[:, gc])
                    g00 = (gp * 2 + gc) * P
                    nc.sync.dma_start(
                        out[g00 : g00 + P, dq * 512 : (dq + 1) * 512], ot
                    )
        op.release()
        w2p.release()
        ps2.release()
        w2p0.release()
        mqp.release()
        dram.release()
    nc.compile()
    return nc


_NC_CACHE = None


def _get_nc():
    global _NC_CACHE
    if _NC_CACHE is None:
        _NC_CACHE = build_nc()
    return _NC_CACHE


def _in_maps(routed_in_egD, w1, w2, w3):
    x = np.ascontiguousarray(np.asarray(routed_in_egD, dtype=np.float32))
    w1 = np.ascontiguousarray(np.asarray(w1, dtype=np.float32))
    w2 = np.ascontiguousarray(np.asarray(w2, dtype=np.float32))
    w3 = np.ascontiguousarray(np.asarray(w3, dtype=np.float32))
    x_e = x.reshape(E, G, D)
    return [
        {"x": x_e[e], "w1": w1[e], "w2": w2[e], "w3": w3[e]} for e in range(E)
    ]


def kernel(routed_in_egD, w1, w2, w3):
    nc = _get_nc()
    in_maps = _in_maps(routed_in_egD, w1, w2, w3)
    try:
        res = run_bass_kernel_spmd(nc, in_maps, core_ids=list(range(E)))
    except Exception:
        # the first execute after process start occasionally dies with a
        # transient NRT_EXEC_UNIT_UNRECOVERABLE through the PJRT tunnel;
        # a straight retry has always succeeded
        res = run_bass_kernel_spmd(nc, in_maps, core_ids=list(range(E)))
    return np.concatenate([r["out"] for r in res.results], axis=0)


def run_traced(routed_in_egD, w1, w2, w3, **trace_kwargs):
    """For test.py: run with NTFF tracing; returns (full_out, BassKernelResults)."""
    nc = _get_nc()
    res = run_bass_kernel_spmd(
        nc,
        _in_maps(routed_in_egD, w1, w2, w3),
        core_ids=list(range(E)),
        trace=True,
        **trace_kwargs,
    )
    out = np.concatenate([r["out"] for r in res.results], axis=0)
    return out, res


# revision 13
# speedup vs baseline: 1.0578x; 1.0578x over previous
"""Expert-parallel SwiGLU MLP (MoE experts) for 8 Trainium2 NeuronCores.

Problem: routed_in_egD [E*G, D] fp32, w1/w3 [E, D, F], w2 [E, F, D], E=8,
G=2048, D=2048, F=5632.  reference:
    x_egD = routed.reshape(E, G, D)
    mid   = silu(x @ w1) * (x @ w3)          # [E, G, F]
    out   = (mid @ w2).reshape(E*G, D)

Sharding: expert-parallel - core e gets expert e's x slice + weights; no
collectives.  Each core runs three 2048x2048x5632-class GEMMs (~142 GFLOP),
matmul-roofline-bound at the bf16 PE rate (1 col/cycle @ 2.4 GHz -> ~1.80 ms
of pure matmul streaming per core), so the whole game is keeping the PE at
~100% MATMUL occupancy: no PE transposes, no phase-boundary stalls.

Per-core kernel (all matmuls bf16, PSUM fp32):
  phase A: SWDGE casts x -> xbf (DRAM, bf16) in 4 g-blocks; HWDGE xbar
           DMA-transposes each block DRAM->SBUF into its own contiguous tile
           xTb[b] [P, DO, 512] (d = do*128 + p row mapping).  SBUF->SBUF
           transposes are NOT used (Tile's deadlock-avoidance serializes them
           to ~10us each); DRAM-sourced transposes run at full rate.  The PE
           does zero transpose work.
  phase 1: gh-outer (g-halves): per (gh, fc): gateT/upT = w1/w3.T @ x
           accumulated over d in PSUM (j-outer so the first 32 matmuls need
           only xTb[0]); SwiGLU (ACT silu -> bf16, DVE mul); midT spilled to
           DRAM bf16 per-gp tiles => precise DMA deps.  w1/w3 are re-read per
           gh (DMA has headroom; PE does not).  w2 dq=0 panel is cast-loaded
           into a pre-allocated buffer DURING gh=0 (interleaved on the SWDGE
           queue), and mq panels for early gp prefetch on the idle sync queue,
           so phase 2 starts with zero DMA wait.
  phase 2: out[g,d] = sum_f midT[f,g]*w2[f,d]: mid panels stationary (bf16),
           w2 panels DMA-cast fp32->bf16 (moving), PSUM accumulation over F.
"""

import numpy as np

import concourse.mybir as mybir
import concourse.tile as tile
from concourse import bacc
from concourse.bass_utils import run_bass_kernel_spmd

E, G, D, F = 8, 2048, 2048, 5632
P = 128
DO = D // P      # 16 d-chunks (contraction steps)
FC = F // P      # 44 f-chunks
GB = 4           # g-blocks of 512 for the x transpose

F32 = mybir.dt.float32
BF16 = mybir.dt.bfloat16


def build_nc():
    nc = bacc.Bacc("TRN2", target_bir_lowering=False)
    x = nc.dram_tensor("x", [G, D], F32, kind="ExternalInput").ap()
    w1 = nc.dram_tensor("w1", [D, F], F32, kind="ExternalInput").ap()
    w2 = nc.dram_tensor("w2", [F, D], F32, kind="ExternalInput").ap()
    w3 = nc.dram_tensor("w3", [D, F], F32, kind="ExternalInput").ap()
    out = nc.dram_tensor("out", [G, D], F32, kind="ExternalOutput").ap()

    # xbar transpose of xbf[g-block, :] into xTb[b] [p, do, g] maps rows as
    # d = do*128 + p (natural d-chunking: partition p within chunk do).
    w1r = w1.rearrange("(do p) f -> p do f", p=P)
    w3r = w3.rearrange("(do p) f -> p do f", p=P)
    w2r = w2.rearrange("(fo p) d -> p fo d", p=P)

    with tile.TileContext(nc) as tc:
        dram = tc.alloc_tile_pool(name="dram", bufs=1, space="DRAM")
        xbf = dram.tile([G, D], BF16, name="xbf")
        # per-gp mid tiles: mids[gp][p, fo, g'] = mid for f = fo*128+p,
        # g = gp*256 + g'.  Phase-1 writes are per-partition contiguous 512B;
        # phase-2 panel reads are per-partition contiguous 22KB.
        mids = [
            dram.tile([P, FC, 256], BF16, tag=f"mid{gp}", name=f"mid{gp}")
            for gp in range(8)
        ]

        # right-side pools pre-allocated so phase-2 inputs stream in during
        # phase 1 (their SBUF space never overlaps the phase-1 pools)
        mqp = tc.alloc_tile_pool(name="mqp", bufs=2, side="right")
        w2p0 = tc.alloc_tile_pool(name="w2p0", bufs=1, side="right")
        w2q0 = w2p0.tile([P, FC, 512], BF16, name="w2q0")

        xtp = tc.alloc_tile_pool(name="xtp", bufs=1)
        xTb = [
            xtp.tile([P, DO, 512], BF16, tag=f"xTb{b}", name=f"xTb{b}")
            for b in range(GB)
        ]

        # ---- phase A: stage x as bf16 in DRAM and xbar-transpose per g-block.
        # HWDGE load (sync) -> DVE cast -> HWDGE store (scalar) -> HWDGE
        # transpose (sync).  The SWDGE queue is left entirely to the w1/w3
        # weight stream (its cast path runs at only ~100-200 GB/s and the
        # first weight tiles gate the first matmuls).
        xfp = tc.alloc_tile_pool(name="xfp", bufs=2)
        xcp = tc.alloc_tile_pool(name="xcp", bufs=2)
        for b in range(GB):
            for s in range(4):
                r = b * 4 + s
                xf = xfp.tile([P, D], F32, tag="xf")
                nc.sync.dma_start(xf, x[r * P : (r + 1) * P, :])
                xc = xcp.tile([P, D], BF16, tag="xc")
                nc.vector.tensor_copy(xc, xf)
                nc.scalar.dma_start(xbf[r * P : (r + 1) * P, :], xc)
            nc.sync.dma_start_transpose(xTb[b], xbf[b * 512 : (b + 1) * 512, :])

        # ---- phase 1: midT[f, g] = silu(w1.T x) * (w3.T x), spill bf16
        wp = tc.alloc_tile_pool(name="wp", bufs=3)
        mp = tc.alloc_tile_pool(name="mp", bufs=3)
        ps1g = tc.alloc_tile_pool(name="ps1g", bufs=2, space="PSUM")
        ps1u = tc.alloc_tile_pool(name="ps1u", bufs=2, space="PSUM")
        w2bounds = [0, 6, 12, 18, 24, 29, 34, 39, 44]

        def p1_mms(pg, pu, w1t, w3t, xs, j):
            for d in range(DO):
                st, sp_ = (d == 0), (d == DO - 1)
                nc.tensor.matmul(pg[:, j], w1t[:, d], xs[:, d], start=st, stop=sp_)
                nc.tensor.matmul(pu[:, j], w3t[:, d], xs[:, d], start=st, stop=sp_)

        def p1_finish(gh, fc, pg, pu):
            mo = mp.tile([P, 4, 256], BF16, tag="mo", name="mo")
            nc.scalar.activation(
                mo, pg.rearrange("p j g -> p (j g)"),
                mybir.ActivationFunctionType.Silu,
            )
            nc.vector.tensor_mul(mo, mo, pu.rearrange("p j g -> p (j g)"))
            for k in range(4):
                nc.scalar.dma_start(mids[gh * 4 + k][:, fc], mo[:, k])

        for gh in range(2):
            grp = {}
            for fc in range(FC):
                w1t = wp.tile([P, DO, P], BF16, tag="w1", name=f"w1t{fc}")
                nc.gpsimd.dma_start(w1t, w1r[:, :, fc * P : (fc + 1) * P])
                w3t = wp.tile([P, DO, P], BF16, tag="w3", name=f"w3t{fc}")
                nc.gpsimd.dma_start(w3t, w3r[:, :, fc * P : (fc + 1) * P])
                if gh == 0 and 4 <= fc < 12:
                    # stream the dq=0 w2 panel into its pre-allocated buffer
                    # while the SWDGE queue has slack
                    lo, hi = w2bounds[fc - 4], w2bounds[fc - 3]
                    nc.gpsimd.dma_start(w2q0[:, lo:hi, :], w2r[:, lo:hi, 0:512])
                pg = ps1g.tile([P, 2, 512], F32, tag="pg")
                pu = ps1u.tile([P, 2, 512], F32, tag="pu")
                if gh == 0 and fc < 2:
                    # xTb[1] is still in flight: run the j=0 halves of fc0 and
                    # fc1 first, then their j=1 halves (same PSUM groups, just
                    # a different fill order), so the PE starts ~20us earlier.
                    p1_mms(pg, pu, w1t, w3t, xTb[0], 0)
                    grp[fc] = (pg, pu, w1t, w3t)
                    if fc == 1:
                        for k in range(2):
                            pgk, puk, w1k, w3k = grp[k]
                            p1_mms(pgk, puk, w1k, w3k, xTb[1], 1)
                            p1_finish(gh, k, pgk, puk)
                    continue
                for j in range(2):
                    p1_mms(pg, pu, w1t, w3t, xTb[gh * 2 + j], j)
                p1_finish(gh, fc, pg, pu)
        mp.release()
        wp.release()
        xcp.release()
        xfp.release()
        xtp.release()
        ps1u.release()
        ps1g.release()

        # ---- phase 2: out[g, d] = midT.T @ w2 (bf16 x bf16, fp32 psum)
        w2p = tc.alloc_tile_pool(name="w2p", bufs=2, side="right")
        op = tc.alloc_tile_pool(name="op", bufs=4, side="right")
        ps2 = tc.alloc_tile_pool(name="ps2", bufs=3, space="PSUM")
        for dq in range(4):
            if dq == 0:
                w2q = w2q0
            else:
                w2q = w2p.tile([P, FC, 512], BF16, tag="w2q")
                nc.gpsimd.dma_start(w2q, w2r[:, :, dq * 512 : (dq + 1) * 512])
            for gp in range(8):
                mq = mqp.tile([P, FC, 256], BF16, tag="mq")
                dma_eng = nc.sync if gp % 2 == 0 else nc.scalar
                dma_eng.dma_start(mq, mids[gp])
                po = ps2.tile([P, 2, 512], F32, tag="po")
                for fo in range(FC):
                    st, sp_ = (fo == 0), (fo == FC - 1)
                    for gc in range(2):
                        nc.tensor.matmul(
                            po[:, gc],
                            mq[:, fo, gc * P : (gc + 1) * P],
                            w2q[:, fo],
                            start=st,
                            stop=sp_,
                        )
                for gc in range(2):
                    ot = op.tile([P, 512], F32, tag="ot")
                    nc.vector.tensor_copy(ot, po[:, gc])
                    g00 = (gp * 2 + gc) * P
                    nc.sync.dma_start(
                        out[g00 : g00 + P, dq * 512 : (dq + 1) * 512], ot
                    )
        op.release()
        w2p.release()
        ps2.release()
        w2p0.release()
        mqp.release()
        dram.release()
    nc.compile()
    return nc


_NC_CACHE = None


def _get_nc():
    global _NC_CACHE
    if _NC_CACHE is None:
        _NC_CACHE = build_nc()
    return _NC_CACHE


def _in_maps(routed_in_egD, w1, w2, w3):
    x = np.ascontiguousarray(np.asarray(routed_in_egD, dtype=np.float32))
    w1 = np.ascontiguousarray(np.asarray(w1, dtype=np.float32))
    w2 = np.ascontiguousarray(np.asarray(w2, dtype=np.float32))
    w3 = np.ascontiguousarray(np.asarray(w3, dtype=np.float32))
    x_e = x.reshape(E, G, D)
    return [
        {"x": x_e[e], "w1": w1[e], "w2": w2[e], "w3": w3[e]} for e in range(E)
    ]


def kernel(routed_in_egD, w1, w2, w3):
    nc = _get_nc()
    in_maps = _in_maps(routed_in_egD, w1, w2, w3)
    try:
        res = run_bass_kernel_spmd(nc, in_maps, core_ids=list(range(E)))
    except Exception:
        # the first execute after process start occasionally dies with a
        # transient NRT_EXEC_UNIT_UNRECOVERABLE through the PJRT tunnel;
        # a straight retry has always succeeded
        res = run_bass_kernel_spmd(nc, in_maps, core_ids=list(range(E)))
    return np.concatenate([r["out"] for r in res.results], axis=0)


def run_traced(routed_in_egD, w1, w2, w3, **trace_kwargs):
    """For test.py: run with NTFF tracing; returns (full_out, BassKernelResults)."""
    nc = _get_nc()
    res = run_bass_kernel_spmd(
        nc,
        _in_maps(routed_in_egD, w1, w2, w3),
        core_ids=list(range(E)),
        trace=True,
        **trace_kwargs,
    )
    out = np.concatenate([r["out"] for r in res.results], axis=0)
    return out, res
